# revision 37
# baseline (speedup 1.0000x reference)
"""FourierBlock Trainium2 kernel.

Math (reference): x = q.transpose(0,2,3,1)  [B,H,E,L]
  X = rfft(x)[..., index]            (gather M selected frequency modes)
  O[b,h,o,m] = sum_i X[b,h,i,m] * (w_real + i*w_imag)[h,i,o,m]
  y = irfft(scatter O into freq slots 0..M-1)

Implemented as three dense matmul stages per (core = head h):
  fwd:  psX[(ri,m), (b,i)] = sum_l Fwd[l,(ri,m)] * q[l,b,i]      (K=l, 16 chunks)
  mix:  psM[(ro,o), (m,b)] = sum_(ri,i) Wbig[m][(ri,i),(ro,o)] * Xt[(ri,i),(m,b)]
  inv:  psY[lp, (b,o)]     = sum_(ri,m) G2[(ri,m), l] * O2[(ri,m),(b,o)]
with DVE 32x32 stream-transposes reshaping between stages.

Sharding: one head per NeuronCore (H=8, 8 cores). Operands are cast to
bf16 on the host (fp32 PSUM accumulation on device); measured end-to-end
relative error ~4e-3.
"""

import os

import numpy as np
import ml_dtypes

import concourse.bacc as bacc
import concourse.mybir as mybir
import concourse.tile as tile
from concourse.bass_utils import run_bass_kernel_spmd

B, L, H, E, M = 32, 2048, 8, 64, 64
NCHUNK = L // 128          # 16 l-chunks of 128
BI = B * E                 # 2048 columns (b, i)
BF16 = mybir.dt.bfloat16
F32 = mybir.dt.float32
NPBF16 = ml_dtypes.bfloat16

_PROGRAM = None


def _build_program():
    nc = bacc.Bacc(target_bir_lowering=False)

    qt = nc.dram_tensor("qt", [NCHUNK, 128, BI], BF16, kind="ExternalInput")
    wb = nc.dram_tensor("wb", [128, M * 128], BF16, kind="ExternalInput")
    fwd = nc.dram_tensor("fwd", [128, NCHUNK * 128], BF16, kind="ExternalInput")
    g2 = nc.dram_tensor("g2", [128, L], BF16, kind="ExternalInput")
    yt = nc.dram_tensor("yt", [NCHUNK, 128, BI], BF16, kind="ExternalOutput")

    with tile.TileContext(nc) as tc:
        with (
            tc.tile_pool(name="const", bufs=1) as cpool,
            tc.tile_pool(name="qpool", bufs=1) as qpool,
            tc.tile_pool(name="work", bufs=1) as wpool,
            tc.tile_pool(name="yout", bufs=3) as ypool,
            tc.tile_pool(name="ps", bufs=2, space="PSUM") as ps,
        ):
            fwd_sb = cpool.tile([128, NCHUNK * 128], BF16, tag="fwd")
            nc.sync.dma_start(out=fwd_sb[:], in_=fwd[:])

            q_bufs = []
            for g in range(NCHUNK // 2):
                qbig = qpool.tile([128, 2 * BI], BF16, tag=f"q{g}")
                nc.sync.dma_start(
                    out=qbig[:].rearrange("p (c f) -> p c f", c=2),
                    in_=qt[2 * g:2 * g + 2].rearrange("c p f -> p c f"),
                )
                q_bufs.append(qbig)
            q_tiles = [
                q_bufs[c // 2][:, (c % 2) * BI:(c % 2 + 1) * BI] for c in range(NCHUNK)
            ]

            g2_sb = cpool.tile([128, L], BF16, tag="g2")
            nc.sync.dma_start(out=g2_sb[:], in_=g2[:])
            wb_sb = cpool.tile([128, M * 128], BF16, tag="wb")
            nc.sync.dma_start(out=wb_sb[:], in_=wb[:])

            # ---- forward DFT: accumulate over the 16 l-chunks ----
            psX = ps.tile([128, BI], F32, tag="acc")

            # Pre-warm the PE clock while the q DMAs stream in: HAM needs
            # ~3.4us of sustained activity to unthrottle 1.2 -> 2.4 GHz.
            for _ in range(16):
                nc.tensor.matmul(psX[:, 0:512], fwd_sb[:, 0:128],
                                 fwd_sb[:, 0:512], start=True, stop=True)
            for c in range(NCHUNK):
                lhsT = fwd_sb[:, c * 128:(c + 1) * 128]
                for j in range(4):
                    nc.tensor.matmul(
                        psX[:, j * 512:(j + 1) * 512],
                        lhsT,
                        q_tiles[c][:, j * 512:(j + 1) * 512],
                        start=(c == 0),
                        stop=(c == NCHUNK - 1),
                    )

            xs_sb = wpool.tile([128, BI], BF16, tag="xs")
            nc.vector.tensor_copy(xs_sb[:], psX[:])

            # Warm-keeper: PE HAM re-throttles to 1.2 GHz after ~3.4 us of
            # idle. During the DVE-bound transpose phase, issue throwaway
            # matmuls gated on freshly produced tiles so the PE clock stays
            # at 2.4 GHz for the inverse stage. They overwrite the already
            # consumed psX region, so no extra PSUM is needed.
            def warm(rhs_ap, n=8):
                k = rhs_ap.partition_size()
                b0 = rhs_ap.base_partition()
                for _ in range(n):
                    nc.tensor.matmul(psX[:, 0:512], fwd_sb[b0:b0 + k, 0:128],
                                     rhs_ap, start=True, stop=True)

            warm(xs_sb[:, 0:512], n=6)

            # ---- transpose 1: [(ri,m), (b,i)] -> [(ri,i), (mB,b,mh)] ----
            # xt2 col layout (mB, b, mh) keeps the transpose WRITES contiguous
            # (scattered SBUF writes cost ~2.5x); the mix reads it strided.
            xt2_sb = wpool.tile([128, M * B], BF16, tag="xt2")
            xs_v = xs_sb[:].rearrange("p (b i) -> p b i", i=E)
            for mB in range(2):
                xt2_v = xt2_sb[:, mB * 1024:(mB + 1) * 1024].rearrange(
                    "p (b m) -> p b m", m=32
                )
                for ri in range(2):
                    for iB in range(2):
                        nc.vector.transpose(
                            xt2_v[ri * 64 + iB * 32: ri * 64 + iB * 32 + 32, :, :],
                            xs_v[ri * 64 + mB * 32: ri * 64 + mB * 32 + 32,
                                 :, iB * 32: iB * 32 + 32],
                        )
                    warm(xt2_sb[ri * 64:(ri + 1) * 64,
                                mB * 1024: mB * 1024 + 512], n=3)

            # ---- mix: per-mode complex channel mixing ----
            psM = ps.tile([128, M * B], F32, tag="acc")
            xt2_r = xt2_sb[:].rearrange("p (q b m) -> p q m b", q=2, b=B)
            for m in range(M):
                nc.tensor.matmul(
                    psM[:, m * B:(m + 1) * B],
                    wb_sb[:, m * 128:(m + 1) * 128],
                    xt2_r[:, m // 32, m % 32, :],
                    start=True,
                    stop=True,
                )

            mixs_sb = wpool.tile([128, M * B], BF16, tag="mixs")
            nc.vector.tensor_copy(mixs_sb[:], psM[:])
            warm(mixs_sb[:, 0:512], n=8)

            # ---- transpose 2: [(ro,o), (m,b)] -> [(ri,m), (oB,b,oh)] ----
            # o2 col layout (oB, b, oh) -> contiguous transpose writes; the
            # host un-permutes the matching output column order for free.
            o2_sb = wpool.tile([128, B * E], BF16, tag="o2")
            mixs_v = mixs_sb[:].rearrange("p (m b) -> p b m", b=B)
            for oB in range(2):
                o2_v = o2_sb[:, oB * 1024:(oB + 1) * 1024].rearrange(
                    "p (b o) -> p b o", o=32
                )
                for r in range(2):
                    for mB in range(2):
                        nc.vector.transpose(
                            o2_v[r * 64 + mB * 32: r * 64 + mB * 32 + 32, :, :],
                            mixs_v[r * 64 + oB * 32: r * 64 + oB * 32 + 32,
                                   :, mB * 32: mB * 32 + 32],
                        )

            # ---- inverse transform + store ----
            # j=0,1 read o2 cols written by the oB=0 transposes, so those
            # matmuls overlap the oB=1 transpose batch. Copies split across
            # Scalar and Vector so both engines drain PSUM concurrently.
            for c in range(NCHUNK):
                psY = ps.tile([128, BI], F32, tag="acc")
                lhsT = g2_sb[:, c * 128:(c + 1) * 128]
                ysb = ypool.tile([128, BI], BF16, tag="y")
                for j in range(4):
                    sl = slice(j * 512, (j + 1) * 512)
                    nc.tensor.matmul(
                        psY[:, sl],
                        lhsT,
                        o2_sb[:, sl],
                        start=True,
                        stop=True,
                    )
                    # quarter-copies right behind each matmul, alternating
                    # engines, so PSUM drains while later j's still multiply
                    if j % 2 == 0:
                        nc.scalar.copy(ysb[:, sl], psY[:, sl])
                    else:
                        nc.vector.tensor_copy(ysb[:, sl], psY[:, sl])
                nc.sync.dma_start(out=yt[c], in_=ysb[:])

    nc.finalize()
    return nc


def _get_program():
    global _PROGRAM
    if _PROGRAM is None:
        _PROGRAM = _build_program()
    return _PROGRAM


def _host_prep(q, w_real, w_imag, index):
    q = np.asarray(q, dtype=np.float32)
    wr = np.asarray(w_real, dtype=np.float32)
    wi = np.asarray(w_imag, dtype=np.float32)
    index = np.asarray(index).astype(np.int64)

    # q -> [H][chunk, p, (b,i)] with l = chunk*128 + p
    qT = np.ascontiguousarray(q.transpose(2, 1, 0, 3))          # [H, L, B, E]
    qt3 = qT.reshape(H, NCHUNK, 128, BI).astype(NPBF16)

    # Wbig[m] = [[wr, wi], [-wi, wr]] laid out [h, (ri,i), (m,(ro,o))]
    wrT = wr.transpose(0, 1, 3, 2)                              # [h, i, m, o]
    wiT = wi.transpose(0, 1, 3, 2)
    A = np.empty((H, 128, M, 128), np.float32)
    A[:, :64, :, :64] = wrT
    A[:, :64, :, 64:] = wiT
    A[:, 64:, :, :64] = -wiT
    A[:, 64:, :, 64:] = wrT
    wb_np = A.reshape(H, 128, M * 128).astype(NPBF16)

    # forward basis: Fwd[l, f] ; f<64 -> cos(2*pi*index[f]*l/L), else -sin
    l = np.arange(L, dtype=np.float64)[:, None]
    ang = 2.0 * np.pi * index[None, :] * l / L                  # [L, M]
    F = np.concatenate([np.cos(ang), -np.sin(ang)], axis=1)     # [L, 2M]
    fwd_np = (
        F.reshape(NCHUNK, 128, 128).transpose(1, 0, 2).reshape(128, NCHUNK * 128)
    ).astype(NPBF16)

    # inverse basis with slot coefficients: slots are 0..M-1
    mm = np.arange(M, dtype=np.float64)
    ang2 = 2.0 * np.pi * mm[:, None] * np.arange(L)[None, :] / L   # [M, L]
    c = np.where(mm == 0, 1.0, 2.0)[:, None] / L
    G2 = np.concatenate([c * np.cos(ang2), -c * np.sin(ang2)], axis=0)  # [2M, L]
    g2_np = G2.astype(NPBF16)

    return qt3, wb_np, fwd_np, g2_np


def run(inputs, trace=False):
    q = inputs["q"]
    qt3, wb_np, fwd_np, g2_np = _host_prep(
        q, inputs["w_real"], inputs["w_imag"], inputs["index"]
    )
    nc = _get_program()
    in_maps = [
        {"qt": qt3[h], "wb": wb_np[h], "fwd": fwd_np, "g2": g2_np}
        for h in range(H)
    ]
    res = run_bass_kernel_spmd(nc, in_maps, list(range(H)), trace=trace)
    arr = np.stack([res.results[h]["yt"] for h in range(H)])    # [H, c, p, (oB,b,oh)]
    arr = arr.astype(np.float32).reshape(H, NCHUNK, 128, 2, B, 32)
    # y[b, h, o=(oB,oh), l=(c,p)]
    y = np.ascontiguousarray(
        arr.transpose(4, 0, 3, 5, 1, 2).reshape(B, H, E, L)
    ).astype(np.float32)
    return y, res


def kernel(**inputs) -> np.ndarray:
    y, _ = run(inputs, trace=False)
    return y
